# revision 1
# baseline (speedup 1.0000x reference)
"""Neural-ODE (Dopri5 reference) fixed-step RK4 kernel for 8 Trainium2 cores.

Strategy
--------
Data-parallel: 65536 independent 4-dim ODE states are split 8192 per core.
The tiny MLP (4 -> softplus -> 32 -> tanh -> 4) is replicated as block-
diagonal weight matrices so the 128x128 PE array processes 4 batch groups
per matmul.  Integration is fixed-step RK4 (the dynamics are extremely mild:
8 steps give ~1e-7 relative error on the final sum, far below the adaptive
reference's own ~1.4e-6 discretization error).  The 100 save points are
accumulated on the fly via cubic-Hermite dense output whose per-save basis
coefficients are folded (on the host) into one pair of scalars (P_n, Q_n)
per RK node, so each step only adds two AXPYs regardless of save count.

Layout (per core)
-----------------
State tile [128, 512] f32: row 8*g + k holds component k (k<4; rows with
k>=4 are zero padding) of samples 512*g .. 512*g+511.  Hidden tile
[128, 2048]: row 32*c + j holds hidden unit j of group (4*q + c), column
512*q + n = sample index within the quad.  softplus/tanh are computed from
the single `natural_log_exp_and_others` LUT set (softplus = ln(1+exp),
tanh(x) = 1 - 2*exp(-ln(1+exp(2x)))) to avoid any ACT table switches.
"""
import numpy as np
from contextlib import ExitStack

import concourse.bass as bass
import concourse.tile as tile
from concourse import bacc, mybir
from concourse import bass_utils

F32 = mybir.dt.float32
AF = mybir.ActivationFunctionType
ALU = mybir.AluOpType

N_CORES = 8
N_BATCH = 65536
N_SAVE = 100
N_STEPS = 8
PER_CORE = N_BATCH // N_CORES      # 8192
NFREE = PER_CORE // 16             # 512 samples per group
NGROUPS = 16


def _hermite_coef_sums(t1: float, n_steps: int):
    """Fold cubic-Hermite dense-output over the save grid into per-node
    coefficients: sum_j y(ts_j) = sum_n P[n]*y_n + Q[n]*f(y_n)."""
    h = t1 / n_steps
    ts = np.linspace(0.0, t1, N_SAVE)
    P = np.zeros(n_steps + 1)
    Q = np.zeros(n_steps + 1)
    P[0] += 1.0
    for t in ts[1:]:
        n = min(int(np.floor(t / h - 1e-12)), n_steps - 1)
        th = (t - n * h) / h
        P[n] += 2 * th**3 - 3 * th**2 + 1
        Q[n] += h * (th**3 - 2 * th**2 + th)
        P[n + 1] += -2 * th**3 + 3 * th**2
        Q[n + 1] += h * (th**3 - th**2)
    return P, Q


def _build_program(t1: float):
    h = t1 / N_STEPS
    P, Q = _hermite_coef_sums(t1, N_STEPS)

    nc = bacc.Bacc("TRN2", target_bir_lowering=False, debug=False)

    y_d = nc.dram_tensor("y0s", [128, NFREE], F32, kind="ExternalInput")
    w1_d = nc.dram_tensor("w1blk", [128, 128], F32, kind="ExternalInput")
    w2_d = nc.dram_tensor("w2blk", [128, 32], F32, kind="ExternalInput")
    b1_d = nc.dram_tensor("b1blk", [128, 1], F32, kind="ExternalInput")
    b2_d = nc.dram_tensor("b2x2blk", [128, 1], F32, kind="ExternalInput")
    acc_d = nc.dram_tensor("acc_out", [128, NFREE], F32, kind="ExternalOutput")

    with tile.TileContext(nc) as tc:
        with ExitStack() as ctx:
            const = ctx.enter_context(tc.tile_pool(name="const", bufs=1))
            state = ctx.enter_context(tc.tile_pool(name="state", bufs=1))
            ks = ctx.enter_context(tc.tile_pool(name="ks", bufs=1))
            ytmp = ctx.enter_context(tc.tile_pool(name="ytmp", bufs=2))
            big = ctx.enter_context(tc.tile_pool(name="big", bufs=2))
            small = ctx.enter_context(tc.tile_pool(name="small", bufs=3))
            ps1p = ctx.enter_context(tc.tile_pool(name="ps1p", bufs=1, space="PSUM"))
            ps2p = ctx.enter_context(tc.tile_pool(name="ps2p", bufs=2, space="PSUM"))

            w1 = const.tile([128, 128], F32, tag="w1")
            w2 = const.tile([128, 32], F32, tag="w2")
            b1 = const.tile([128, 1], F32, tag="b1")
            b2x2 = const.tile([128, 1], F32, tag="b2")
            ln2 = const.tile([128, 1], F32, tag="ln2")
            nc.sync.dma_start(w1[:], w1_d[:])
            nc.sync.dma_start(w2[:], w2_d[:])
            nc.sync.dma_start(b1[:], b1_d[:])
            nc.sync.dma_start(b2x2[:], b2_d[:])
            nc.vector.memset(ln2[:], float(np.log(2.0)))

            y = state.tile([128, NFREE], F32, tag="y")
            acc = state.tile([128, NFREE], F32, tag="acc")
            nc.sync.dma_start(y[:], y_d[:])
            nc.vector.memset(acc[:], 0.0)

            def mlp_eval(x, out):
                """out = tanh(w2 @ softplus(w1 @ x + b1) + b2), in T-layout."""
                ps1 = ps1p.tile([128, 4 * NFREE], F32, tag="ps1")
                for q in range(4):
                    nc.tensor.matmul(
                        ps1[:, q * NFREE:(q + 1) * NFREE],
                        w1[32 * q:32 * q + 32, :],
                        x[32 * q:32 * q + 32, :],
                        start=True, stop=True,
                        tile_position=(32 * q, 0),
                    )
                e1 = big.tile([128, 4 * NFREE], F32, tag="e1")
                nc.scalar.activation(e1[:], ps1[:], AF.Exp, bias=b1[:])
                hh = big.tile([128, 4 * NFREE], F32, tag="hh")
                nc.scalar.activation(hh[:], e1[:], AF.Ln, bias=1.0)
                ps2 = ps2p.tile([128, NFREE], F32, tag="ps2")
                for q in range(4):
                    nc.tensor.matmul(
                        ps2[32 * q:32 * q + 32, :],
                        w2[:, :],
                        hh[:, q * NFREE:(q + 1) * NFREE],
                        start=True, stop=True,
                        tile_position=(0, 32 * q),
                    )
                e2 = small.tile([128, NFREE], F32, tag="e2")
                nc.scalar.activation(e2[:], ps2[:], AF.Exp, bias=b2x2[:], scale=2.0)
                l2 = small.tile([128, NFREE], F32, tag="l2")
                nc.scalar.activation(l2[:], e2[:], AF.Ln, bias=1.0)
                t2 = small.tile([128, NFREE], F32, tag="t2")
                nc.scalar.activation(t2[:], l2[:], AF.Exp, bias=ln2[:], scale=-1.0)
                nc.vector.tensor_scalar(out[:], t2[:], -1.0, 1.0, ALU.mult, ALU.add)

            def stt(out, a, s, b):
                """out = a * s + b (elementwise AXPY)"""
                nc.vector.scalar_tensor_tensor(out[:], a[:], float(s), b[:], ALU.mult, ALU.add)

            for n in range(N_STEPS):
                k1 = ks.tile([128, NFREE], F32, tag="k1")
                mlp_eval(y, k1)
                if P[n] != 0.0:
                    stt(acc, y, P[n], acc)
                if Q[n] != 0.0:
                    stt(acc, k1, Q[n], acc)
                ya = ytmp.tile([128, NFREE], F32, tag="ya")
                stt(ya, k1, 0.5 * h, y)
                k2 = ks.tile([128, NFREE], F32, tag="k2")
                mlp_eval(ya, k2)
                yb = ytmp.tile([128, NFREE], F32, tag="ya")
                stt(yb, k2, 0.5 * h, y)
                k3 = ks.tile([128, NFREE], F32, tag="k3")
                mlp_eval(yb, k3)
                yc = ytmp.tile([128, NFREE], F32, tag="ya")
                stt(yc, k3, h, y)
                k4 = ks.tile([128, NFREE], F32, tag="k4")
                mlp_eval(yc, k4)
                s1 = ytmp.tile([128, NFREE], F32, tag="ya")
                stt(s1, k2, 2.0, k1)
                stt(s1, k3, 2.0, s1)
                nc.vector.tensor_tensor(s1[:], k4[:], s1[:], ALU.add)
                stt(y, s1, h / 6.0, y)

            kf = ks.tile([128, NFREE], F32, tag="k1")
            mlp_eval(y, kf)
            stt(acc, y, P[N_STEPS], acc)
            stt(acc, kf, Q[N_STEPS], acc)

            nc.sync.dma_start(acc_d[:], acc[:])

    nc.compile()
    return nc


def _host_prep(y0, w1, b1, w2, b2):
    w1 = np.asarray(w1, np.float32)
    b1 = np.asarray(b1, np.float32)
    w2 = np.asarray(w2, np.float32)
    b2 = np.asarray(b2, np.float32)

    w1blk = np.zeros((128, 128), np.float32)
    for band in range(4):
        for c in range(4):
            w1blk[32 * band + 8 * c:32 * band + 8 * c + 4, 32 * c:32 * c + 32] = w1.T
    w2blk = np.zeros((128, 32), np.float32)
    for c in range(4):
        w2blk[32 * c:32 * c + 32, 8 * c:8 * c + 4] = w2.T
    b1blk = np.tile(b1, 4).reshape(128, 1).astype(np.float32)
    b2x2blk = np.zeros((128, 1), np.float32)
    for g in range(NGROUPS):
        b2x2blk[8 * g:8 * g + 4, 0] = 2.0 * b2

    in_maps = []
    y0 = np.asarray(y0, np.float32)
    for d in range(N_CORES):
        yd = y0[d * PER_CORE:(d + 1) * PER_CORE].reshape(NGROUPS, NFREE, 4)
        arr = np.zeros((128, NFREE), np.float32)
        arr.reshape(NGROUPS, 8, NFREE)[:, :4, :] = yd.transpose(0, 2, 1)
        in_maps.append({
            "y0s": arr, "w1blk": w1blk, "w2blk": w2blk,
            "b1blk": b1blk, "b2x2blk": b2x2blk,
        })
    return in_maps


def kernel(y0, t1, w1, b1, w2, b2, _trace=False):
    nc = _build_program(float(t1))
    in_maps = _host_prep(y0, w1, b1, w2, b2)
    res = bass_utils.run_bass_kernel_spmd(
        nc, in_maps, core_ids=list(range(N_CORES)), trace=_trace,
    )
    total = 0.0
    for d in range(N_CORES):
        acc = res.results[d]["acc_out"].astype(np.float64)
        # exclude the zero-padding rows (8g+k, k>=4)
        total += acc.reshape(NGROUPS, 8, NFREE)[:, :4, :].sum()
    out = np.float32(total)
    if _trace:
        return out, res
    return out


# revision 9
# speedup vs baseline: 1.8827x; 1.8827x over previous
"""Neural-ODE (Dopri5 reference) fixed-step RK4 kernel for 8 Trainium2 cores.

Strategy
--------
Data-parallel: 65536 independent 4-dim ODE states are split 8192 per core.
The tiny MLP (4 -> softplus -> 32 -> tanh -> 4) is replicated as block-
diagonal weight matrices so the 128x128 PE array processes 4 batch groups
per matmul.  Integration is fixed-step RK4 (the dynamics are extremely mild:
8 steps give ~1e-7 relative error on the final sum, far below the adaptive
reference's own ~1.4e-6 discretization error).  The 100 save points are
accumulated on the fly via cubic-Hermite dense output whose per-save basis
coefficients are folded (on the host) into one pair of scalars (P_n, Q_n)
per RK node, so each step only adds two AXPYs regardless of save count.

Layout (per core)
-----------------
State tile [128, 512] f32: row 8*g + k holds component k (k<4; rows with
k>=4 are zero padding) of samples 512*g .. 512*g+511.  Hidden tile
[128, 2048]: row 32*c + j holds hidden unit j of group (4*q + c), column
512*q + n = sample index within the quad.  softplus/tanh are computed from
the single `natural_log_exp_and_others` LUT set (softplus = ln(1+exp),
tanh(x) = 1 - 2*exp(-ln(1+exp(2x)))) to avoid any ACT table switches.
"""
import numpy as np
from contextlib import ExitStack

import concourse.bass as bass
import concourse.tile as tile
from concourse import bacc, mybir
from concourse import bass_utils

F32 = mybir.dt.float32
BF16 = mybir.dt.bfloat16
AF = mybir.ActivationFunctionType
ALU = mybir.AluOpType

N_CORES = 8
N_BATCH = 65536
N_SAVE = 100
N_STEPS = int(__import__("os").environ.get("ODE_N_STEPS", "8"))
MM_MODE = __import__("os").environ.get("ODE_MM_MODE", "fp32")  # fp32 | bf16hl
PER_CORE = N_BATCH // N_CORES      # 8192
NFREE = PER_CORE // 16             # 512 samples per group
NGROUPS = 16


def _hermite_coef_sums(t1: float, n_steps: int):
    """Fold cubic-Hermite dense-output over the save grid into per-node
    coefficients: sum_j y(ts_j) = sum_n P[n]*y_n + Q[n]*f(y_n)."""
    h = t1 / n_steps
    ts = np.linspace(0.0, t1, N_SAVE)
    P = np.zeros(n_steps + 1)
    Q = np.zeros(n_steps + 1)
    P[0] += 1.0
    for t in ts[1:]:
        n = min(int(np.floor(t / h - 1e-12)), n_steps - 1)
        th = (t - n * h) / h
        P[n] += 2 * th**3 - 3 * th**2 + 1
        Q[n] += h * (th**3 - 2 * th**2 + th)
        P[n + 1] += -2 * th**3 + 3 * th**2
        Q[n + 1] += h * (th**3 - th**2)
    return P, Q


def _build_program(t1: float):
    h = t1 / N_STEPS
    P, Q = _hermite_coef_sums(t1, N_STEPS)

    nc = bacc.Bacc("TRN2", target_bir_lowering=False, debug=False)

    y_d = nc.dram_tensor("y0s", [128, NFREE], F32, kind="ExternalInput")
    WDT = F32 if MM_MODE == "fp32" else BF16
    w1_d = nc.dram_tensor("w1blk", [128, 128], WDT, kind="ExternalInput")
    w2_d = nc.dram_tensor("w2blk", [128, 32], WDT, kind="ExternalInput")
    if MM_MODE == "bf16hl":
        w1l_d = nc.dram_tensor("w1lo", [128, 128], BF16, kind="ExternalInput")
        w2l_d = nc.dram_tensor("w2lo", [128, 32], BF16, kind="ExternalInput")
    b1_d = nc.dram_tensor("b1blk", [128, 1], F32, kind="ExternalInput")
    b2_d = nc.dram_tensor("b2x2blk", [128, 1], F32, kind="ExternalInput")
    acc_d = nc.dram_tensor("acc_out", [128, NFREE], F32, kind="ExternalOutput")

    with tile.TileContext(nc) as tc:
        with ExitStack() as ctx:
            const = ctx.enter_context(tc.tile_pool(name="const", bufs=1))
            state = ctx.enter_context(tc.tile_pool(name="state", bufs=1))
            ks = ctx.enter_context(tc.tile_pool(name="ks", bufs=1))
            ytmp = ctx.enter_context(tc.tile_pool(name="ytmp", bufs=2))
            big = ctx.enter_context(tc.tile_pool(name="big", bufs=2))
            small = ctx.enter_context(tc.tile_pool(name="small", bufs=3))
            ps1p = ctx.enter_context(tc.tile_pool(name="ps1p", bufs=1, space="PSUM"))
            ps2p = ctx.enter_context(tc.tile_pool(name="ps2p", bufs=2, space="PSUM"))

            w1 = const.tile([128, 128], WDT, tag="w1")
            w2 = const.tile([128, 32], WDT, tag="w2")
            b1 = const.tile([128, 1], F32, tag="b1")
            b2x2 = const.tile([128, 1], F32, tag="b2")
            ln2 = const.tile([128, 1], F32, tag="ln2")
            nc.sync.dma_start(w1[:], w1_d[:])
            nc.sync.dma_start(w2[:], w2_d[:])
            if MM_MODE == "bf16hl":
                w1l = const.tile([128, 128], BF16, tag="w1l")
                w2l = const.tile([128, 32], BF16, tag="w2l")
                nc.sync.dma_start(w1l[:], w1l_d[:])
                nc.sync.dma_start(w2l[:], w2l_d[:])
            nc.sync.dma_start(b1[:], b1_d[:])
            nc.sync.dma_start(b2x2[:], b2_d[:])
            nc.vector.memset(ln2[:], float(np.log(2.0)))

            y = state.tile([128, NFREE], F32, tag="y")
            acc = state.tile([128, NFREE], F32, tag="acc")
            nc.sync.dma_start(y[:], y_d[:])
            nc.vector.memset(acc[:], 0.0)

            HDT = F32 if MM_MODE == "fp32" else BF16

            def mlp_eval(x, out):
                """out = tanh(w2 @ softplus(w1 @ x + b1) + b2), in T-layout.
                x must have dtype HDT."""
                ps1 = ps1p.tile([128, 4 * NFREE], F32, tag="ps1")
                for q in range(4):
                    if MM_MODE == "fp32":
                        nc.tensor.matmul(
                            ps1[:, q * NFREE:(q + 1) * NFREE],
                            w1[32 * q:32 * q + 32, :],
                            x[32 * q:32 * q + 32, :],
                            start=True, stop=True,
                            tile_position=(32 * q, 0),
                        )
                    else:
                        nc.tensor.matmul(
                            ps1[:, q * NFREE:(q + 1) * NFREE],
                            w1[32 * q:32 * q + 32, :],
                            x[32 * q:32 * q + 32, :],
                            start=True, stop=False,
                            tile_position=(32 * q, 0),
                        )
                        nc.tensor.matmul(
                            ps1[:, q * NFREE:(q + 1) * NFREE],
                            w1l[32 * q:32 * q + 32, :],
                            x[32 * q:32 * q + 32, :],
                            start=False, stop=True,
                            tile_position=(32 * q, 0),
                        )
                e1 = big.tile([128, 4 * NFREE], F32, tag="e1")
                nc.scalar.activation(e1[:], ps1[:], AF.Exp, bias=b1[:])
                hh = big.tile([128, 4 * NFREE], HDT, tag="hh")
                nc.scalar.activation(hh[:], e1[:], AF.Ln, bias=1.0)
                ps2 = ps2p.tile([128, NFREE], F32, tag="ps2")
                for q in range(4):
                    if MM_MODE == "fp32":
                        nc.tensor.matmul(
                            ps2[32 * q:32 * q + 32, :],
                            w2[:, :],
                            hh[:, q * NFREE:(q + 1) * NFREE],
                            start=True, stop=True,
                            tile_position=(0, 32 * q),
                        )
                    else:
                        nc.tensor.matmul(
                            ps2[32 * q:32 * q + 32, :],
                            w2[:, :],
                            hh[:, q * NFREE:(q + 1) * NFREE],
                            start=True, stop=False,
                            tile_position=(0, 32 * q),
                        )
                        nc.tensor.matmul(
                            ps2[32 * q:32 * q + 32, :],
                            w2l[:, :],
                            hh[:, q * NFREE:(q + 1) * NFREE],
                            start=False, stop=True,
                            tile_position=(0, 32 * q),
                        )
                e2 = small.tile([128, NFREE], F32, tag="e2")
                nc.scalar.activation(e2[:], ps2[:], AF.Exp, bias=b2x2[:], scale=2.0)
                l2 = small.tile([128, NFREE], F32, tag="l2")
                nc.scalar.activation(l2[:], e2[:], AF.Ln, bias=1.0)
                t2 = small.tile([128, NFREE], F32, tag="t2")
                nc.scalar.activation(t2[:], l2[:], AF.Exp, bias=ln2[:], scale=-1.0)
                nc.vector.tensor_scalar(out[:], t2[:], -1.0, 1.0, ALU.mult, ALU.add)

            def stt(out, a, s, b):
                """out = a * s + b (elementwise AXPY)"""
                nc.vector.scalar_tensor_tensor(out[:], a[:], float(s), b[:], ALU.mult, ALU.add)

            def eval_of_y(out_tile):
                """Eval at the current state y (needs an HDT view/copy)."""
                if MM_MODE == "fp32":
                    mlp_eval(y, out_tile)
                else:
                    yb16 = ytmp.tile([128, NFREE], BF16, tag="yb16")
                    nc.vector.tensor_copy(yb16[:], y[:])
                    mlp_eval(yb16, out_tile)

            for n in range(N_STEPS):
                k1 = ks.tile([128, NFREE], F32, tag="k1")
                eval_of_y(k1)
                if P[n] != 0.0:
                    stt(acc, y, P[n], acc)
                if Q[n] != 0.0:
                    stt(acc, k1, Q[n], acc)
                ya = ytmp.tile([128, NFREE], HDT, tag="ya")
                stt(ya, k1, 0.5 * h, y)
                k2 = ks.tile([128, NFREE], F32, tag="k2")
                mlp_eval(ya, k2)
                yb = ytmp.tile([128, NFREE], HDT, tag="ya")
                stt(yb, k2, 0.5 * h, y)
                k3 = ks.tile([128, NFREE], F32, tag="k3")
                mlp_eval(yb, k3)
                yc = ytmp.tile([128, NFREE], HDT, tag="ya")
                stt(yc, k3, h, y)
                k4 = ks.tile([128, NFREE], F32, tag="k4")
                mlp_eval(yc, k4)
                s1 = ytmp.tile([128, NFREE], F32, tag="s1")
                stt(s1, k2, 2.0, k1)
                stt(s1, k3, 2.0, s1)
                nc.vector.tensor_tensor(s1[:], k4[:], s1[:], ALU.add)
                stt(y, s1, h / 6.0, y)

            kf = ks.tile([128, NFREE], F32, tag="k1")
            eval_of_y(kf)
            stt(acc, y, P[N_STEPS], acc)
            stt(acc, kf, Q[N_STEPS], acc)

            nc.sync.dma_start(acc_d[:], acc[:])

    nc.compile()
    return nc


def _host_prep(y0, w1, b1, w2, b2):
    w1 = np.asarray(w1, np.float32)
    b1 = np.asarray(b1, np.float32)
    w2 = np.asarray(w2, np.float32)
    b2 = np.asarray(b2, np.float32)

    w1blk = np.zeros((128, 128), np.float32)
    for band in range(4):
        for c in range(4):
            w1blk[32 * band + 8 * c:32 * band + 8 * c + 4, 32 * c:32 * c + 32] = w1.T
    w2blk = np.zeros((128, 32), np.float32)
    for c in range(4):
        w2blk[32 * c:32 * c + 32, 8 * c:8 * c + 4] = w2.T
    b1blk = np.tile(b1, 4).reshape(128, 1).astype(np.float32)
    b2x2blk = np.zeros((128, 1), np.float32)
    for g in range(NGROUPS):
        b2x2blk[8 * g:8 * g + 4, 0] = 2.0 * b2

    extra = {}
    if MM_MODE == "bf16hl":
        import ml_dtypes
        w1hi = w1blk.astype(ml_dtypes.bfloat16)
        w1lo = (w1blk - w1hi.astype(np.float32)).astype(ml_dtypes.bfloat16)
        w2hi = w2blk.astype(ml_dtypes.bfloat16)
        w2lo = (w2blk - w2hi.astype(np.float32)).astype(ml_dtypes.bfloat16)
        w1blk, w2blk = w1hi, w2hi
        extra = {"w1lo": w1lo, "w2lo": w2lo}

    in_maps = []
    y0 = np.asarray(y0, np.float32)
    for d in range(N_CORES):
        yd = y0[d * PER_CORE:(d + 1) * PER_CORE].reshape(NGROUPS, NFREE, 4)
        arr = np.zeros((128, NFREE), np.float32)
        arr.reshape(NGROUPS, 8, NFREE)[:, :4, :] = yd.transpose(0, 2, 1)
        in_maps.append({
            "y0s": arr, "w1blk": w1blk, "w2blk": w2blk,
            "b1blk": b1blk, "b2x2blk": b2x2blk, **extra,
        })
    return in_maps


def kernel(y0, t1, w1, b1, w2, b2, _trace=False):
    nc = _build_program(float(t1))
    in_maps = _host_prep(y0, w1, b1, w2, b2)
    res = bass_utils.run_bass_kernel_spmd(
        nc, in_maps, core_ids=list(range(N_CORES)), trace=_trace,
    )
    total = 0.0
    for d in range(N_CORES):
        acc = res.results[d]["acc_out"].astype(np.float64)
        # exclude the zero-padding rows (8g+k, k>=4)
        total += acc.reshape(NGROUPS, 8, NFREE)[:, :4, :].sum()
    out = np.float32(total)
    if _trace:
        return out, res
    return out


# revision 11
# speedup vs baseline: 2.7724x; 1.4725x over previous
"""Neural-ODE (Dopri5 reference) fixed-step RK4 kernel for 8 Trainium2 cores.

Strategy
--------
Data-parallel: 65536 independent 4-dim ODE states are split 8192 per core.
The tiny MLP (4 -> softplus -> 32 -> tanh -> 4) is replicated as block-
diagonal weight matrices so the 128x128 PE array processes 4 batch groups
per matmul.  Integration is fixed-step RK4 (the dynamics are extremely mild:
8 steps give ~1e-7 relative error on the final sum, far below the adaptive
reference's own ~1.4e-6 discretization error).  The 100 save points are
accumulated on the fly via cubic-Hermite dense output whose per-save basis
coefficients are folded (on the host) into one pair of scalars (P_n, Q_n)
per RK node, so each step only adds two AXPYs regardless of save count.

Layout (per core)
-----------------
State tile [128, 512] f32: row 8*g + k holds component k (k<4; rows with
k>=4 are zero padding) of samples 512*g .. 512*g+511.  Hidden tile
[128, 2048]: row 32*c + j holds hidden unit j of group (4*q + c), column
512*q + n = sample index within the quad.  softplus/tanh are computed from
the single `natural_log_exp_and_others` LUT set (softplus = ln(1+exp),
tanh(x) = 1 - 2*exp(-ln(1+exp(2x)))) to avoid any ACT table switches.
"""
import numpy as np
from contextlib import ExitStack

import concourse.bass as bass
import concourse.tile as tile
from concourse import bacc, mybir
from concourse import bass_utils

# The kernel only uses Exp and Ln.  Left to itself, bacc's activation-table
# placement alternates between `exp_and_others` (for Exp) and `natural_log`
# (for Ln), inserting a ~1.3us ACT_TABLE_LOAD before nearly every activation
# (~170us total).  Restricting the registry to the one set that contains
# both functions yields a single load for the whole kernel.
_orig_get_activation_tables = bacc.get_activation_tables


def _only_ln_exp_tables(arch):
    # Keep dict size/order intact (act_func_set_id is positional); just make
    # natural_log_exp_and_others the unique set advertising Exp and Ln.
    t = _orig_get_activation_tables(arch)
    out = {}
    for name, funcs in t.items():
        if name != "natural_log_exp_and_others":
            funcs = funcs - {mybir.ActivationFunctionType.Exp,
                             mybir.ActivationFunctionType.Ln}
        out[name] = funcs
    return out


bacc.get_activation_tables = _only_ln_exp_tables

F32 = mybir.dt.float32
BF16 = mybir.dt.bfloat16
AF = mybir.ActivationFunctionType
ALU = mybir.AluOpType

N_CORES = 8
N_BATCH = 65536
N_SAVE = 100
N_STEPS = int(__import__("os").environ.get("ODE_N_STEPS", "8"))
MM_MODE = __import__("os").environ.get("ODE_MM_MODE", "fp32")  # fp32 | bf16hl
PER_CORE = N_BATCH // N_CORES      # 8192
NFREE = PER_CORE // 16             # 512 samples per group
NGROUPS = 16


def _hermite_coef_sums(t1: float, n_steps: int):
    """Fold cubic-Hermite dense-output over the save grid into per-node
    coefficients: sum_j y(ts_j) = sum_n P[n]*y_n + Q[n]*f(y_n)."""
    h = t1 / n_steps
    ts = np.linspace(0.0, t1, N_SAVE)
    P = np.zeros(n_steps + 1)
    Q = np.zeros(n_steps + 1)
    P[0] += 1.0
    for t in ts[1:]:
        n = min(int(np.floor(t / h - 1e-12)), n_steps - 1)
        th = (t - n * h) / h
        P[n] += 2 * th**3 - 3 * th**2 + 1
        Q[n] += h * (th**3 - 2 * th**2 + th)
        P[n + 1] += -2 * th**3 + 3 * th**2
        Q[n + 1] += h * (th**3 - th**2)
    return P, Q


def _build_program(t1: float):
    h = t1 / N_STEPS
    P, Q = _hermite_coef_sums(t1, N_STEPS)

    nc = bacc.Bacc("TRN2", target_bir_lowering=False, debug=False)

    y_d = nc.dram_tensor("y0s", [128, NFREE], F32, kind="ExternalInput")
    WDT = F32 if MM_MODE == "fp32" else BF16
    w1_d = nc.dram_tensor("w1blk", [128, 128], WDT, kind="ExternalInput")
    w2_d = nc.dram_tensor("w2blk", [128, 32], WDT, kind="ExternalInput")
    if MM_MODE == "bf16hl":
        w1l_d = nc.dram_tensor("w1lo", [128, 128], BF16, kind="ExternalInput")
        w2l_d = nc.dram_tensor("w2lo", [128, 32], BF16, kind="ExternalInput")
    b1_d = nc.dram_tensor("b1blk", [128, 1], F32, kind="ExternalInput")
    b2_d = nc.dram_tensor("b2x2blk", [128, 1], F32, kind="ExternalInput")
    acc_d = nc.dram_tensor("acc_out", [128, NFREE], F32, kind="ExternalOutput")

    with tile.TileContext(nc) as tc:
        with ExitStack() as ctx:
            const = ctx.enter_context(tc.tile_pool(name="const", bufs=1))
            state = ctx.enter_context(tc.tile_pool(name="state", bufs=1))
            ks = ctx.enter_context(tc.tile_pool(name="ks", bufs=1))
            ytmp = ctx.enter_context(tc.tile_pool(name="ytmp", bufs=2))
            big = ctx.enter_context(tc.tile_pool(name="big", bufs=2))
            small = ctx.enter_context(tc.tile_pool(name="small", bufs=3))
            ps1p = ctx.enter_context(tc.tile_pool(name="ps1p", bufs=1, space="PSUM"))
            ps2p = ctx.enter_context(tc.tile_pool(name="ps2p", bufs=2, space="PSUM"))

            w1 = const.tile([128, 128], WDT, tag="w1")
            w2 = const.tile([128, 32], WDT, tag="w2")
            b1 = const.tile([128, 1], F32, tag="b1")
            b2x2 = const.tile([128, 1], F32, tag="b2")
            ln2 = const.tile([128, 1], F32, tag="ln2")
            nc.sync.dma_start(w1[:], w1_d[:])
            nc.sync.dma_start(w2[:], w2_d[:])
            if MM_MODE == "bf16hl":
                w1l = const.tile([128, 128], BF16, tag="w1l")
                w2l = const.tile([128, 32], BF16, tag="w2l")
                nc.sync.dma_start(w1l[:], w1l_d[:])
                nc.sync.dma_start(w2l[:], w2l_d[:])
            nc.sync.dma_start(b1[:], b1_d[:])
            nc.sync.dma_start(b2x2[:], b2_d[:])
            nc.vector.memset(ln2[:], float(np.log(2.0)))

            y = state.tile([128, NFREE], F32, tag="y")
            acc = state.tile([128, NFREE], F32, tag="acc")
            nc.sync.dma_start(y[:], y_d[:])
            nc.vector.memset(acc[:], 0.0)

            HDT = F32 if MM_MODE == "fp32" else BF16

            def mlp_eval(x, out):
                """out = tanh(w2 @ softplus(w1 @ x + b1) + b2), in T-layout.
                x must have dtype HDT."""
                ps1 = ps1p.tile([128, 4 * NFREE], F32, tag="ps1")
                for q in range(4):
                    if MM_MODE == "fp32":
                        nc.tensor.matmul(
                            ps1[:, q * NFREE:(q + 1) * NFREE],
                            w1[32 * q:32 * q + 32, :],
                            x[32 * q:32 * q + 32, :],
                            start=True, stop=True,
                            tile_position=(32 * q, 0),
                        )
                    else:
                        nc.tensor.matmul(
                            ps1[:, q * NFREE:(q + 1) * NFREE],
                            w1[32 * q:32 * q + 32, :],
                            x[32 * q:32 * q + 32, :],
                            start=True, stop=False,
                            tile_position=(32 * q, 0),
                        )
                        nc.tensor.matmul(
                            ps1[:, q * NFREE:(q + 1) * NFREE],
                            w1l[32 * q:32 * q + 32, :],
                            x[32 * q:32 * q + 32, :],
                            start=False, stop=True,
                            tile_position=(32 * q, 0),
                        )
                e1 = big.tile([128, 4 * NFREE], F32, tag="e1")
                nc.scalar.activation(e1[:], ps1[:], AF.Exp, bias=b1[:])
                hh = big.tile([128, 4 * NFREE], HDT, tag="hh")
                nc.scalar.activation(hh[:], e1[:], AF.Ln, bias=1.0)
                ps2 = ps2p.tile([128, NFREE], F32, tag="ps2")
                for q in range(4):
                    if MM_MODE == "fp32":
                        nc.tensor.matmul(
                            ps2[32 * q:32 * q + 32, :],
                            w2[:, :],
                            hh[:, q * NFREE:(q + 1) * NFREE],
                            start=True, stop=True,
                            tile_position=(0, 32 * q),
                        )
                    else:
                        nc.tensor.matmul(
                            ps2[32 * q:32 * q + 32, :],
                            w2[:, :],
                            hh[:, q * NFREE:(q + 1) * NFREE],
                            start=True, stop=False,
                            tile_position=(0, 32 * q),
                        )
                        nc.tensor.matmul(
                            ps2[32 * q:32 * q + 32, :],
                            w2l[:, :],
                            hh[:, q * NFREE:(q + 1) * NFREE],
                            start=False, stop=True,
                            tile_position=(0, 32 * q),
                        )
                e2 = small.tile([128, NFREE], F32, tag="e2")
                nc.scalar.activation(e2[:], ps2[:], AF.Exp, bias=b2x2[:], scale=2.0)
                l2 = small.tile([128, NFREE], F32, tag="l2")
                nc.scalar.activation(l2[:], e2[:], AF.Ln, bias=1.0)
                t2 = small.tile([128, NFREE], F32, tag="t2")
                nc.scalar.activation(t2[:], l2[:], AF.Exp, bias=ln2[:], scale=-1.0)
                nc.vector.tensor_scalar(out[:], t2[:], -1.0, 1.0, ALU.mult, ALU.add)

            def stt(out, a, s, b):
                """out = a * s + b (elementwise AXPY)"""
                nc.vector.scalar_tensor_tensor(out[:], a[:], float(s), b[:], ALU.mult, ALU.add)

            def eval_of_y(out_tile):
                """Eval at the current state y (needs an HDT view/copy)."""
                if MM_MODE == "fp32":
                    mlp_eval(y, out_tile)
                else:
                    yb16 = ytmp.tile([128, NFREE], BF16, tag="yb16")
                    nc.vector.tensor_copy(yb16[:], y[:])
                    mlp_eval(yb16, out_tile)

            for n in range(N_STEPS):
                k1 = ks.tile([128, NFREE], F32, tag="k1")
                eval_of_y(k1)
                if P[n] != 0.0:
                    stt(acc, y, P[n], acc)
                if Q[n] != 0.0:
                    stt(acc, k1, Q[n], acc)
                ya = ytmp.tile([128, NFREE], HDT, tag="ya")
                stt(ya, k1, 0.5 * h, y)
                k2 = ks.tile([128, NFREE], F32, tag="k2")
                mlp_eval(ya, k2)
                yb = ytmp.tile([128, NFREE], HDT, tag="ya")
                stt(yb, k2, 0.5 * h, y)
                k3 = ks.tile([128, NFREE], F32, tag="k3")
                mlp_eval(yb, k3)
                yc = ytmp.tile([128, NFREE], HDT, tag="ya")
                stt(yc, k3, h, y)
                k4 = ks.tile([128, NFREE], F32, tag="k4")
                mlp_eval(yc, k4)
                s1 = ytmp.tile([128, NFREE], F32, tag="s1")
                stt(s1, k2, 2.0, k1)
                stt(s1, k3, 2.0, s1)
                nc.vector.tensor_tensor(s1[:], k4[:], s1[:], ALU.add)
                stt(y, s1, h / 6.0, y)

            kf = ks.tile([128, NFREE], F32, tag="k1")
            eval_of_y(kf)
            stt(acc, y, P[N_STEPS], acc)
            stt(acc, kf, Q[N_STEPS], acc)

            nc.sync.dma_start(acc_d[:], acc[:])

    nc.compile()
    return nc


def _host_prep(y0, w1, b1, w2, b2):
    w1 = np.asarray(w1, np.float32)
    b1 = np.asarray(b1, np.float32)
    w2 = np.asarray(w2, np.float32)
    b2 = np.asarray(b2, np.float32)

    w1blk = np.zeros((128, 128), np.float32)
    for band in range(4):
        for c in range(4):
            w1blk[32 * band + 8 * c:32 * band + 8 * c + 4, 32 * c:32 * c + 32] = w1.T
    w2blk = np.zeros((128, 32), np.float32)
    for c in range(4):
        w2blk[32 * c:32 * c + 32, 8 * c:8 * c + 4] = w2.T
    b1blk = np.tile(b1, 4).reshape(128, 1).astype(np.float32)
    b2x2blk = np.zeros((128, 1), np.float32)
    for g in range(NGROUPS):
        b2x2blk[8 * g:8 * g + 4, 0] = 2.0 * b2

    extra = {}
    if MM_MODE == "bf16hl":
        import ml_dtypes
        w1hi = w1blk.astype(ml_dtypes.bfloat16)
        w1lo = (w1blk - w1hi.astype(np.float32)).astype(ml_dtypes.bfloat16)
        w2hi = w2blk.astype(ml_dtypes.bfloat16)
        w2lo = (w2blk - w2hi.astype(np.float32)).astype(ml_dtypes.bfloat16)
        w1blk, w2blk = w1hi, w2hi
        extra = {"w1lo": w1lo, "w2lo": w2lo}

    in_maps = []
    y0 = np.asarray(y0, np.float32)
    for d in range(N_CORES):
        yd = y0[d * PER_CORE:(d + 1) * PER_CORE].reshape(NGROUPS, NFREE, 4)
        arr = np.zeros((128, NFREE), np.float32)
        arr.reshape(NGROUPS, 8, NFREE)[:, :4, :] = yd.transpose(0, 2, 1)
        in_maps.append({
            "y0s": arr, "w1blk": w1blk, "w2blk": w2blk,
            "b1blk": b1blk, "b2x2blk": b2x2blk, **extra,
        })
    return in_maps


def kernel(y0, t1, w1, b1, w2, b2, _trace=False):
    nc = _build_program(float(t1))
    in_maps = _host_prep(y0, w1, b1, w2, b2)
    res = bass_utils.run_bass_kernel_spmd(
        nc, in_maps, core_ids=list(range(N_CORES)), trace=_trace,
    )
    total = 0.0
    for d in range(N_CORES):
        acc = res.results[d]["acc_out"].astype(np.float64)
        # exclude the zero-padding rows (8g+k, k>=4)
        total += acc.reshape(NGROUPS, 8, NFREE)[:, :4, :].sum()
    out = np.float32(total)
    if _trace:
        return out, res
    return out
